# revision 20
# baseline (speedup 1.0000x reference)
"""Causal self-attention (B=2, T=2048, C=1024, H=16) on 8 TRN2 NeuronCores.

Sharding: core = (batch b, head-group hg) with b in {0,1}, hg in {0..3};
each core computes Q/K/V projections and attention for its 4 heads on its
batch, plus the row-parallel slice of the output projection. The host sums
the 4 per-core partial projections per batch (bf16) and adds the output bias.

Device algorithm (all layouts transposed so softmax needs no on-chip
transposes):
  - x and weights are pre-gathered HOST-side into the exact SBUF layouts so
    every DMA line is >=1KB contiguous (~340 GB/s instead of ~210); tg=0's x
    and the pair-0 weights are split in half so the first chains start after
    ~0.3 MB; a burst of tiny warm-up matmuls on a memset tile (no DMA dep)
    releases the HAM clock throttle before the first real chain.
  - Q^T, K^T [dd, t] and V [t, dd] via bf16 matmul chains (contraction over C).
  - S^T[s, t]: 2 heads row-packed in the PE array (K=64 at row offsets 0/64).
  - exp on ScalarE straight out of PSUM (scale=1/sqrt(d) folded in); causal
    masking = one additive 128x128 band on diagonal blocks + trimming the
    S matmul, the exp AND the AV matmul to the unmasked column range;
    softmax denominators from an all-ones column appended to V (M=65
    matmul); normalization deferred to after AV. The tail block's
    reciprocal comes from a K=1 all-ones broadcast matmul + DVE reciprocal
    in PSUM (no DRAM bounce).
  - softmax reciprocals: denominator row bounced through DRAM into a
    [128, 8] partition-major tile, one DVE reciprocal op (ScalarE runs
    exps only), bounced back and broadcast-read for the normalize muls.
  - Emission interleaves projection/QKV chains into the attention blocks'
    spare PE slots (attention is ScalarE-exp paced) so the PE never idles.
  - y_partial[t, e] bf16 out; host sums partials in f32.
"""

import math
from collections import deque
from functools import lru_cache

import ml_dtypes
import numpy as np

import concourse.bass as bass
import concourse.mybir as mybir
from concourse import bacc
import concourse.tile as tile
from concourse import bass_utils

F32 = mybir.dt.float32
BF16 = mybir.dt.bfloat16
EXP = mybir.ActivationFunctionType.Exp

B, T, C, H = 2, 2048, 1024, 16
NCORES = 8
NH = 4            # heads per core
D = C // H        # 64
DD = NH * D       # 256 channels per core
P = 128
TG = 512          # t-group width (matmul moving dim)
NG = T // TG      # 4
NT = T // P       # 16 s-chunks
CCH = C // P      # 8 contraction chunks
NEG = -8.0e6      # pre-scale additive mask; *0.125 = -1e6 like the reference

LAST_RESULTS = None  # BassKernelResults of the most recent run (for test.py)


class FillerQueue:
    """PE work units interleaved into the attention blocks' spare slots."""

    def __init__(self):
        self.q = deque()

    def add(self, units):
        self.q.extend(units)

    def add_front(self, units):
        for u in reversed(units):
            self.q.appendleft(u)

    def pump(self, n=1):
        for _ in range(n):
            if not self.q:
                return
            self.q.popleft()()

    def flush(self):
        while self.q:
            self.q.popleft()()


def build_program(apply_kbias: bool, general_mask: bool) -> bass.Bass:
    nc = bacc.Bacc("TRN2", target_bir_lowering=False, debug=False,
                   enable_asserts=False)

    # host pre-gathered layouts (contiguous 1-8 KB DMA lines; the on-device
    # strided gathers ran the input stream at ~210 GB/s instead of ~340)
    xg = nc.dram_tensor("xg", [NG * P, CCH * TG], BF16, kind="ExternalInput").ap()
    wqg = nc.dram_tensor("wqg", [2 * P, CCH * P], BF16, kind="ExternalInput").ap()
    wkg = nc.dram_tensor("wkg", [2 * P, CCH * P], BF16, kind="ExternalInput").ap()
    wvg = nc.dram_tensor("wvg", [P, CCH * DD], BF16, kind="ExternalInput").ap()
    wpT = nc.dram_tensor("wpT", [DD, C], BF16, kind="ExternalInput").ap()
    bqk = nc.dram_tensor("bqk", [P, 4], F32, kind="ExternalInput").ap()
    bv_in = nc.dram_tensor("bv_sb", [P, DD], F32, kind="ExternalInput").ap()
    ident_in = nc.dram_tensor("ident", [D, D], BF16, kind="ExternalInput").ap()
    kbias_in = None
    if apply_kbias:
        kbias_in = nc.dram_tensor("kbias", [P, NT], F32, kind="ExternalInput").ap()
    band_in = maskT = None
    if general_mask:
        maskT = nc.dram_tensor("maskT", [T, T], F32, kind="ExternalInput").ap()
    else:
        band_in = nc.dram_tensor("band", [P, P], F32, kind="ExternalInput").ap()
    yp = nc.dram_tensor("yp", [T, C], BF16, kind="ExternalOutput").ap()
    # DRAM bounce buffers for the softmax denominators: raw rows land in
    # rcd_raw, get re-read [128, 8] partition-major (contiguous 8-elem lines),
    # reciprocated on DVE, written back t-major to rcd_rcp, then broadcast
    # across partitions (DMA from DRAM may use a 0-step partition dim).
    rcd_raw = nc.dram_tensor("rcd_raw", [2 * NG, 2 * TG], BF16, kind="Internal").ap()
    rcd_rcp = nc.dram_tensor("rcd_rcp", [2 * NG, 2 * TG], BF16, kind="Internal").ap()

    with tile.TileContext(nc) as tc:
        with tc.tile_pool(name="wts", bufs=1) as wts, \
             tc.tile_pool(name="xtp", bufs=1) as xtp, \
             tc.tile_pool(name="qkv", bufs=1) as qkv, \
             tc.tile_pool(name="otp", bufs=1) as otp, \
             tc.tile_pool(name="ptp", bufs=4) as ptp, \
             tc.tile_pool(name="asb", bufs=4) as asbp, \
             tc.tile_pool(name="rtp", bufs=2) as rtp, \
             tc.tile_pool(name="bcp", bufs=2) as bcp, \
             tc.tile_pool(name="tmp", bufs=3) as tmpp, \
             tc.tile_pool(name="ysb", bufs=6) as ysbp, \
             tc.tile_pool(name="mkp", bufs=2) as mkp, \
             tc.tile_pool(name="stp", bufs=2, space="PSUM") as stp, \
             tc.tile_pool(name="avp", bufs=2, space="PSUM") as avp, \
             tc.tile_pool(name="mmp", bufs=2, space="PSUM") as mmp:

            # Only Exp (and friends) are needed; preload so the act-table
            # DMA overlaps the input DMAs instead of stalling the first exp.
            from concourse.hw_specs import get_activation_tables
            tables = get_activation_tables(nc.m.arch)
            set_id = list(tables).index("natural_log_exp_and_others")
            nc.scalar.add_instruction(mybir.InstLoadActFuncSet(
                name=nc.get_next_instruction_name(), ins=[], outs=[],
                act_func_set_id=set_id))

            # ---- all-ones tile: warm-up stationary operand (no DMA dep so
            # the warm-up starts right after the preamble) + the K=1
            # broadcast matmul of the tail block's softmax denominators.
            ones_t = wts.tile([P, D], BF16, name="ones_t")
            nc.gpsimd.memset(ones_t, 1.0)

            # ---- input DMAs ----
            # gpsimd queue: x (t-group-major, first half of tg=0 leads) +
            # the small f32 tables; sync queue: weights, Q before K (the
            # chains run Q first) with each pair-0 weight split in half so
            # the first chain units start after ~0.25 MB.
            bqk_t = wts.tile([P, 4], F32, name="bqk_t")

            wkh = [[wts.tile([P, 4 * P], BF16, name=f"wkh{i}{h}")
                    for h in range(2)] for i in range(2)]
            wqh = [[wts.tile([P, 4 * P], BF16, name=f"wqh{i}{h}")
                    for h in range(2)] for i in range(2)]
            wvh = [wts.tile([P, 4 * DD], BF16, name=f"wvh{h}") for h in range(2)]
            xt0 = [xtp.tile([P, 4 * TG], BF16, name=f"xt0{h}") for h in range(2)]
            xtg = [None] + [xtp.tile([P, CCH * TG], BF16, name=f"xtg{t_}")
                            for t_ in range(1, NG)]

            # x stream on the gpsimd queue; pair-0 Q/K weights on sync; the
            # V weights + small tables on the scalar queue and the pair-1
            # weights on the vector queue so descriptor issue + transfers of
            # the first block's inputs all run in parallel.
            nc.gpsimd.dma_start(out=xt0[0], in_=xg[0:P, 0:4 * TG])
            nc.gpsimd.dma_start(out=xt0[1], in_=xg[0:P, 4 * TG:8 * TG])
            for t_ in range(1, NG):
                nc.gpsimd.dma_start(out=xtg[t_], in_=xg[t_ * P:(t_ + 1) * P, :])

            for h in range(2):
                nc.sync.dma_start(
                    out=wqh[0][h], in_=wqg[0:P, h * 4 * P:(h + 1) * 4 * P])
            for h in range(2):
                nc.sync.dma_start(
                    out=wkh[0][h], in_=wkg[0:P, h * 4 * P:(h + 1) * 4 * P])
            nc.sync.dma_start(out=bqk_t, in_=bqk)
            if kbias_in is not None:
                kbias_t = wts.tile([P, NT], F32, name="kbias_t")
                nc.sync.dma_start(out=kbias_t, in_=kbias_in)

            bv_sb = wts.tile([P, DD], F32, name="bv_t")
            band_t = None
            if band_in is not None:
                band_t = wts.tile([P, P], F32, name="band_t")
                nc.scalar.dma_start(out=band_t, in_=band_in)
            ident_t = wts.tile([D, D], BF16, name="ident_t")
            nc.scalar.dma_start(out=ident_t, in_=ident_in)
            nc.scalar.dma_start(out=bv_sb, in_=bv_in)
            for h in range(2):
                nc.scalar.dma_start(
                    out=wvh[h], in_=wvg[0:P, h * 4 * DD:(h + 1) * 4 * DD])

            for h in range(2):
                nc.sync.dma_start(
                    out=wqh[1][h], in_=wqg[P:2 * P, h * 4 * P:(h + 1) * 4 * P])
            for h in range(2):
                nc.sync.dma_start(
                    out=wkh[1][h], in_=wkg[P:2 * P, h * 4 * P:(h + 1) * 4 * P])
            wp = [wts.tile([P, C], BF16, name=f"wp{i}") for i in range(2)]
            for i in range(2):
                nc.scalar.dma_start(out=wp[i], in_=wpT[i * P:(i + 1) * P, :])

            qt = [qkv.tile([P, T], BF16, name=f"qt{i}") for i in range(2)]
            kt = [qkv.tile([P, T], BF16, name=f"kt{i}") for i in range(2)]
            vaug = [qkv.tile([P, NH * (D + 1)], BF16, name=f"vaug{j}")
                    for j in range(NT)]
            ot = [otp.tile([P, T], BF16, name=f"ot{i}") for i in range(2)]

            # ---- HAM warm-up: ~3us of tiny matmuls so the PE clock is at
            # 8/8 by the time the first projection chain lands.
            # N=64 so 96 matmuls span ~5us of continuous PE activity (the
            # HAM SHORT window needs >=3.4us busy to unthrottle the clock)
            wps = mmp.tile([P, TG], F32, name="mm", tag="mm")
            for _ in range(96):
                nc.tensor.matmul(wps[0:1, 0:D], lhsT=ones_t[:, 0:1],
                                 rhs=ones_t[:, 0:D], start=True, stop=True)

            # ---- QKV chain units ----
            def wsl(whalves, c, iw):
                return whalves[iw][c // 4][:, (c % 4) * P:(c % 4 + 1) * P]

            def xsl(tg, c):
                if tg == 0:
                    return xt0[c // 4][:, (c % 4) * TG:(c % 4 + 1) * TG]
                return xtg[tg][:, c * TG:(c + 1) * TG]

            def xvsl(j, c):
                tg, u = divmod(j, NG)
                base = xsl(tg, c)
                return base[:, u * P:u * P + P]

            def qk_chain_units(iw, tg):
                """K then Q projection chain for dd-tile iw, t-group tg.
                Split into 2-matmul units + a trailing bias/drain unit (the
                drain lags its chain by one pump slot to avoid head-of-line
                blocking on the DVE queue)."""
                units = []
                for wall, dst, bcol in ((wqh, qt, iw), (wkh, kt, 2 + iw)):
                    box = {}

                    def mk_mm(c0, wall=wall, box=box):
                        def f():
                            if c0 == 0:
                                box['ps'] = mmp.tile([P, TG], F32, name="mm",
                                                     tag="mm")
                            for c in (c0, c0 + 1):
                                nc.tensor.matmul(
                                    box['ps'], lhsT=wsl(wall, c, iw),
                                    rhs=xsl(tg, c),
                                    start=(c == 0), stop=(c == CCH - 1))
                        return f

                    def mk_bias(dst=dst, bcol=bcol, box=box):
                        def f():
                            nc.vector.tensor_scalar_add(
                                dst[iw][:, tg * TG:(tg + 1) * TG], box['ps'],
                                bqk_t[:, bcol:bcol + 1])
                        return f

                    units += [mk_mm(0), mk_mm(2), mk_mm(4), mk_mm(6),
                              mk_bias()]
                return units

            def v_chain_units(j):
                box = {}

                def mk_mm(c0):
                    def f():
                        if c0 == 0:
                            box['ps'] = mmp.tile([P, TG], F32, name="mm",
                                                 tag="mm")
                        for c in range(c0, c0 + 4):
                            nc.tensor.matmul(
                                box['ps'][:, :DD], lhsT=xvsl(j, c),
                                rhs=wvh[c // 4][:, (c % 4) * DD:(c % 4 + 1) * DD],
                                start=(c == 0), stop=(c == CCH - 1))
                    return f

                def drain():
                    ps = box['ps']
                    vview = vaug[j].rearrange("p (h x) -> p h x", h=NH)
                    bvv = bv_sb.rearrange("p (h x) -> p h x", h=NH)
                    # ones column (softmax denominator row): in0*0 + 1
                    nc.vector.tensor_scalar(
                        vview[:, :, D:D + 1], bvv[:, :, 0:1], 0.0, 1.0,
                        mybir.AluOpType.mult, mybir.AluOpType.add)
                    nc.vector.tensor_add(
                        vview[:, :, 0:D],
                        ps[:, :DD].rearrange("p (h x) -> p h x", h=NH), bvv)

                return [mk_mm(0), mk_mm(4), drain]

            def proj_units(tt, ec):
                box = {}
                alt = (tt * 2 + ec) % 2

                def mm():
                    box['ps'] = mmp.tile([P, TG], F32, name="mm", tag="mm")
                    for i2 in range(2):
                        nc.tensor.matmul(
                            box['ps'], lhsT=ot[i2][:, tt * P:(tt + 1) * P],
                            rhs=wp[i2][:, ec * TG:(ec + 1) * TG],
                            start=(i2 == 0), stop=(i2 == 1))

                def drain():
                    # alternate the PSUM->SBUF cast and the store DMA across
                    # engines/queues so the drains pipeline 2-wide
                    ysb = ysbp.tile([P, TG], BF16, name="ysb", tag="ysb")
                    if alt:
                        nc.scalar.activation(
                            ysb, box['ps'], mybir.ActivationFunctionType.Copy)
                        nc.gpsimd.dma_start(
                            out=yp[tt * P:(tt + 1) * P,
                                   ec * TG:(ec + 1) * TG], in_=ysb)
                    else:
                        nc.vector.tensor_copy(ysb, box['ps'])
                        nc.sync.dma_start(
                            out=yp[tt * P:(tt + 1) * P,
                                   ec * TG:(ec + 1) * TG], in_=ysb)

                return [mm, drain]

            # ---- attention ----
            def attn_block(i, g, fq, pending=None, defer_av=False,
                           fast_recip=False, late_fq=None, tail_units=None):
                # causal: only s-chunks on/below the diagonal contribute.
                # `pending` is the previous block's deferred epilogue tail,
                # emitted after S(1) so its DMA-bounce waits never block this
                # block's DVE stream. `defer_av` emits all S's before any AV
                # (first block: the V chains feeding AV are still in fq).
                nj = NT if general_mask else 4 * g + 4
                av = [avp.tile([P, TG], F32, name="av", tag="av")
                      for _ in range(2)]
                pump_n = 2 if nj == 8 else 3

                def pump(n):
                    for _ in range(n):
                        if late_fq is not None and late_fq.q:
                            late_fq.q.popleft()()
                        elif fq.q:
                            fq.q.popleft()()

                def emit_S(j):
                    r = j - 4 * g
                    # causal trim: s-chunk j only attends to t >= r*P within
                    # this t-group; skip the masked-out left columns in the
                    # S matmul AND the exp (the AV matmul already trims the
                    # same range).
                    trim = r * P if (r >= 1 and not general_mask) else 0
                    st = stp.tile([P, 2 * TG], F32, name="st", tag="st")
                    for h in range(2):
                        nc.tensor.matmul(
                            st[:, h * TG + trim:(h + 1) * TG],
                            lhsT=(kt[i][64 * h:64 * h + 64,
                                        j * P:(j + 1) * P]),
                            rhs=(qt[i][64 * h:64 * h + 64,
                                       g * TG + trim:(g + 1) * TG]),
                            start=True, stop=True,
                            tile_position=(64 * h, 0))
                    if general_mask:
                        mk = mkp.tile([P, TG], F32, name="mk", tag="mk")
                        nc.sync.dma_start(
                            out=mk,
                            in_=maskT[j * P:(j + 1) * P, g * TG:(g + 1) * TG])
                        for h in range(2):
                            nc.vector.tensor_add(
                                st[:, h * TG:(h + 1) * TG],
                                st[:, h * TG:(h + 1) * TG], mk)
                    elif r >= 0:
                        for h in range(2):
                            sl = slice(h * TG + r * P, h * TG + (r + 1) * P)
                            nc.vector.tensor_add(st[:, sl], st[:, sl], band_t)
                    if apply_kbias:
                        for h in range(2):
                            nc.vector.tensor_scalar_add(
                                st[:, h * TG + trim:(h + 1) * TG],
                                st[:, h * TG + trim:(h + 1) * TG],
                                kbias_t[:, j:j + 1])
                    pt = ptp.tile([P, 2 * TG], BF16, name="pt", tag="pt")
                    if trim:
                        stv = st.rearrange("p (h t) -> p h t", h=2)
                        ptv = pt.rearrange("p (h t) -> p h t", h=2)
                        nc.scalar.activation(ptv[:, :, trim:], stv[:, :, trim:],
                                             EXP, scale=1.0 / math.sqrt(D))
                    else:
                        nc.scalar.activation(pt, st, EXP,
                                             scale=1.0 / math.sqrt(D))
                    return pt

                def emit_AV(j, pt):
                    r = j - 4 * g
                    trim = r * P if (r > 0 and not general_mask) else 0
                    for h in range(2):
                        nc.tensor.matmul(
                            av[h][0:D + 1, trim:TG],
                            lhsT=(vaug[j][:, (2 * i + h) * (D + 1):
                                          (2 * i + h + 1) * (D + 1)]),
                            rhs=(pt[:, h * TG + trim:(h + 1) * TG]),
                            start=(j == 0), stop=(j == nj - 1),
                            skip_group_check=True)

                prev = None
                for j in range(nj):
                    pt = emit_S(j)
                    if j == 1 and pending is not None:
                        pending()
                        pending = None
                    if prev is not None and not defer_av:
                        if late_fq is not None and prev[0] == nj - 4:
                            late_fq.flush()
                        emit_AV(*prev)
                    if not defer_av:
                        prev = (j, pt)
                    else:
                        prev = prev or []
                        prev.append((j, pt))
                    if j >= 1:
                        pump(pump_n)
                if defer_av:
                    fq.flush()
                    for j, pt in prev:
                        emit_AV(j, pt)
                else:
                    if late_fq is not None and prev[0] == nj - 4:
                        late_fq.flush()
                    emit_AV(*prev)

                # Epilogue part A: free the accumulator banks, launch the
                # denominator row into the DRAM transpose bounce.
                slot = i * NG + g
                asb = asbp.tile([D + 1, 2 * TG], BF16, name="asb", tag="asb")
                nc.vector.tensor_copy(asb[:, 0:TG], av[0][0:D + 1, :])
                nc.vector.tensor_copy(asb[:, TG:2 * TG], av[1][0:D + 1, :])

                def normalize(bc):
                    # upper-half (tm) first: its extra DMA hop into ot is on
                    # the critical path of the following projection
                    tm = tmpp.tile([P, TG], BF16, name="tm", tag="tm")
                    nc.vector.tensor_mul(tm[0:D, :], asb[0:D, TG:2 * TG],
                                         bc[0:D, TG:2 * TG])
                    nc.sync.dma_start(
                        out=ot[i][D:P, g * TG:(g + 1) * TG],
                        in_=tm[0:D, :])
                    nc.vector.tensor_mul(
                        ot[i][0:D, g * TG:(g + 1) * TG],
                        asb[0:D, 0:TG], bc[0:D, 0:TG])

                if fast_recip:
                    # Tail block. The softmax denominator row (asb row D,
                    # already in SBUF from the bank-freeing copies) is
                    # broadcast across partitions 0..63 with a K=1 all-ones
                    # matmul into a free PSUM pair, reciprocated on the DVE -
                    # no DRAM bounce, ~6us less dead latency than the old
                    # ln/exp + double-DMA path. Meanwhile the final
                    # projection's accumulation chains OPEN with their
                    # ot[0]-half (ready since phase 1) across the free PSUM
                    # banks, and CLOSE with the ot[1]-half after normalize.
                    units = [(t_, e_) for t_ in range(4 * g, 4 * g + 4)
                             for e_ in range(2)]
                    tp = []
                    st_t = stp.tile([P, 2 * TG], F32, name="st", tag="st")

                    def open_unit(bi, tt, ec):
                        if bi < 2:
                            ps = st_t[:, bi * TG:(bi + 1) * TG]
                        elif bi < 4:
                            ps = avp.tile([P, TG], F32, name="av", tag="av")
                        else:
                            ps = mmp.tile([P, TG], F32, name="mm", tag="mm")
                        nc.tensor.matmul(
                            ps, lhsT=ot[0][:, tt * P:(tt + 1) * P],
                            rhs=wp[0][:, ec * TG:(ec + 1) * TG],
                            start=True, stop=False, skip_group_check=True)
                        tp.append((tt, ec, ps))

                    for bi, (tt, ec) in enumerate(units[:4]):
                        open_unit(bi, tt, ec)
                    # denominator broadcast across partitions (K=1 matmul on
                    # the all-ones column) + fast DVE reciprocal from PSUM
                    bcps = stp.tile([P, 2 * TG], F32, name="st", tag="st")
                    for h in range(2):
                        nc.tensor.matmul(
                            bcps[0:D, h * TG:(h + 1) * TG],
                            lhsT=ones_t[D:D + 1, 0:D],
                            rhs=asb[D:D + 1, h * TG:(h + 1) * TG],
                            start=True, stop=True)
                    # 2 more opens issued behind the broadcast keep the PE
                    # warm through the reciprocal+normalize window
                    for bi in (4, 5):
                        open_unit(bi, *units[bi])
                    bc = bcp.tile([P, 2 * TG], F32, name="bcf", tag="bcf")
                    # ~51-ULP Newton-Raphson reciprocal: 5x faster than the
                    # exact DVE reciprocal (which measured 6.5us on [64,1024])
                    nc.vector.reciprocal_approx_fast(bc[0:D, :], bcps[0:D, :])
                    # normalize; the heads-2/3 half moves to partitions 64..127
                    # via an identity matmul into bcps' upper partitions + a
                    # DVE copy instead of the ~2.5us SBUF->SBUF DMA round trip
                    tm = tmpp.tile([P, TG], BF16, name="tm", tag="tm")
                    nc.vector.tensor_mul(tm[0:D, :], asb[0:D, TG:2 * TG],
                                         bc[0:D, TG:2 * TG])
                    nc.tensor.matmul(
                        bcps[D:P, 0:TG], lhsT=ident_t, rhs=tm[0:D, :],
                        start=True, stop=True)
                    nc.vector.tensor_copy(ot[i][D:P, g * TG:(g + 1) * TG],
                                          bcps[D:P, 0:TG])
                    nc.vector.tensor_mul(
                        ot[i][0:D, g * TG:(g + 1) * TG],
                        asb[0:D, 0:TG], bc[0:D, 0:TG])

                    def pdrain(bi, tt, ec, ps):
                        # casts alternate scalar/vector; stores all on the
                        # sync ring (the gpsimd ring's flush was the last
                        # thing to finish in the teardown)
                        ysb = ysbp.tile([P, TG], BF16, name="ysb", tag="ysb")
                        if bi % 2:
                            nc.scalar.activation(
                                ysb, ps, mybir.ActivationFunctionType.Copy)
                        else:
                            nc.vector.tensor_copy(ysb, ps)
                        nc.sync.dma_start(
                            out=yp[tt * P:(tt + 1) * P,
                                   ec * TG:(ec + 1) * TG], in_=ysb)

                    for bi, (tt, ec, ps) in enumerate(tp):
                        nc.tensor.matmul(
                            ps, lhsT=ot[1][:, tt * P:(tt + 1) * P],
                            rhs=wp[1][:, ec * TG:(ec + 1) * TG],
                            start=False, stop=True, skip_group_check=True)
                        pdrain(bi, tt, ec, ps)
                    # last 2 units had no free PSUM bank for an early open
                    # (bcps holds 2 banks); run them as plain pairs
                    for bi, (tt, ec) in enumerate(units[6:]):
                        ps = mmp.tile([P, TG], F32, name="mm", tag="mm")
                        for i2 in range(2):
                            nc.tensor.matmul(
                                ps, lhsT=ot[i2][:, tt * P:(tt + 1) * P],
                                rhs=wp[i2][:, ec * TG:(ec + 1) * TG],
                                start=(i2 == 0), stop=(i2 == 1))
                        pdrain(bi, tt, ec, ps)
                    return None

                nc.gpsimd.dma_start(out=rcd_raw[slot], in_=asb[D:D + 1, :])
                rt = rtp.tile([P, 8], BF16, name="rt", tag="rt")
                nc.gpsimd.dma_start(out=rt, in_=bass.AP(
                    tensor=rcd_raw.tensor, offset=rcd_raw[slot].offset,
                    ap=[[8, P], [1, 8]]))

                # Epilogue part B (deferred into the next block so the
                # bounce round-trips never stall this DVE/sync stream).
                def part_b():
                    rw = rtp.tile([P, 8], BF16, name="rw", tag="rw")
                    with nc.allow_low_precision(
                            reason="bf16 softmax denominators (~0.4% rel)"):
                        nc.vector.reciprocal(rw, rt)
                    nc.gpsimd.dma_start(out=bass.AP(
                        tensor=rcd_rcp.tensor, offset=rcd_rcp[slot].offset,
                        ap=[[8, P], [1, 8]]), in_=rw)
                    bc = bcp.tile([P, 2 * TG], BF16, name="bc", tag="bc")
                    nc.gpsimd.dma_start(out=bc[0:D, :], in_=bass.AP(
                        tensor=rcd_rcp.tensor, offset=rcd_rcp[slot].offset,
                        ap=[[0, D], [1, 2 * TG]]))
                    normalize(bc)

                return part_b

            # ---- emission schedule ----
            # Pair 0's first chains + V(0..3) run before its g=0 block; the
            # rest of QKV, pair-1 chains and the output projections are fed
            # through the filler queue into the attention blocks' spare PE
            # slots (attention is exp-paced on ScalarE).
            fq = FillerQueue()
            fq.add(qk_chain_units(0, 0))
            fq.flush()
            for j in range(NT if general_mask else 4):
                fq.add(v_chain_units(j))
            pend = None
            cur_late = None
            for g in range(NG):
                if g < NG - 1:
                    fq.add(qk_chain_units(0, g + 1))
                    nxt_late = None
                    if not general_mask:
                        nxt_late = FillerQueue()
                        for j in range(4 * (g + 1), 4 * (g + 2)):
                            nxt_late.add(v_chain_units(j))
                else:
                    fq.add(qk_chain_units(1, 0))
                    nxt_late = None
                pend = attn_block(0, g, fq, pending=pend, defer_av=(g == 0),
                                  late_fq=cur_late)
                fq.flush()
                cur_late = nxt_late
            for g in range(NG):
                if g < NG - 1:
                    fq.add_front(qk_chain_units(1, g + 1))
                pend = attn_block(1, g, fq, pending=pend,
                                  fast_recip=(g == NG - 1))
                fq.flush()
                if g < NG - 1:
                    for tt in range(4 * g, 4 * g + 4):
                        for ec in range(2):
                            fq.add(proj_units(tt, ec))
            if pend is not None:
                pend()
            fq.flush()

    nc.compile()
    return nc


@lru_cache(maxsize=4)
def _program(apply_kbias: bool, general_mask: bool) -> bass.Bass:
    return build_program(apply_kbias, general_mask)


def _host_prep(inputs):
    x = np.asarray(inputs["x"], np.float32)
    Wq = np.asarray(inputs["Wq"], np.float32)
    bq = np.asarray(inputs["bq"], np.float32)
    Wk = np.asarray(inputs["Wk"], np.float32)
    bk = np.asarray(inputs["bk"], np.float32)
    Wv = np.asarray(inputs["Wv"], np.float32)
    bv = np.asarray(inputs["bv"], np.float32)
    Wp = np.asarray(inputs["Wp"], np.float32)
    attn_mask = np.asarray(inputs["attn_mask"])
    valid = np.asarray(inputs["valid_input_mask"])

    tril = np.tril(np.ones((T, T), attn_mask.dtype))
    causal = all(np.array_equal(attn_mask[b], tril) for b in range(B))
    kbias_all = (valid.astype(np.float32) - 1.0) * 1e6  # [B, T]
    apply_kbias = bool((valid == 0).any())

    band = np.where(np.arange(P)[:, None] <= np.arange(P)[None, :],
                    np.float32(0.0), np.float32(NEG))

    # device-ready layouts (pre-gathered so every DMA line is >=1 KB
    # contiguous):
    #   xg[tg*P+p, c*TG+u]   = x[b][tg*TG+u, c*P+p]
    #   wqg[iw*P+p, c*P+m]   = Wq[sl][iw*P+m, c*P+p]   (same for wk)
    #   wvg[p, c*DD+q]       = Wv[sl][q, c*P+p]
    def xg_prep(xb):
        return np.ascontiguousarray(
            xb.reshape(NG, TG, CCH, P).transpose(0, 3, 2, 1)
            .reshape(NG * P, CCH * TG)).astype(ml_dtypes.bfloat16)

    def wqkg_prep(Wsl):
        return np.ascontiguousarray(
            Wsl.reshape(2, P, CCH, P).transpose(0, 3, 2, 1)
            .reshape(2 * P, CCH * P)).astype(ml_dtypes.bfloat16)

    def wvg_prep(Wsl):
        return np.ascontiguousarray(
            Wsl.reshape(DD, CCH, P).transpose(2, 1, 0)
            .reshape(P, CCH * DD)).astype(ml_dtypes.bfloat16)

    in_maps = []
    for core in range(NCORES):
        b, hg = divmod(core, 4)
        sl = slice(hg * DD, (hg + 1) * DD)
        m = {
            "xg": xg_prep(x[b]),
            "wqg": wqkg_prep(Wq[sl, :]),
            "wkg": wqkg_prep(Wk[sl, :]),
            "wvg": wvg_prep(Wv[sl, :]),
            "wpT": np.ascontiguousarray(Wp[:, sl].T).astype(ml_dtypes.bfloat16),
            "bqk": np.ascontiguousarray(
                np.stack([bq[sl][:P], bq[sl][P:], bk[sl][:P], bk[sl][P:]], 1)),
            "bv_sb": np.ascontiguousarray(np.tile(bv[sl], (P, 1))),
            "ident": np.eye(D, dtype=ml_dtypes.bfloat16),
        }
        if apply_kbias:
            m["kbias"] = np.ascontiguousarray(kbias_all[b].reshape(NT, P).T)
        if not causal:
            m["maskT"] = np.ascontiguousarray(
                (attn_mask[b].T.astype(np.float32) - 1.0) * (-NEG))
        else:
            m["band"] = band
        in_maps.append(m)
    return in_maps, apply_kbias, causal


def _run(inputs, trace=False, trace_cores=None):
    global LAST_RESULTS
    in_maps, apply_kbias, causal = _host_prep(inputs)
    nc = _program(apply_kbias, not causal)
    res = bass_utils.run_bass_kernel_spmd(
        nc, in_maps, core_ids=list(range(NCORES)), trace=trace,
        trace_cores=trace_cores)
    LAST_RESULTS = res

    bp = np.asarray(inputs["bp"], np.float32)
    y = np.zeros((B, T, C), np.float32)
    for core in range(NCORES):
        y[core // 4] += np.asarray(res.results[core]["yp"], np.float32)
    y += bp[None, None, :]
    return y


def kernel(**inputs) -> np.ndarray:
    return _run(inputs)



# revision 23
# speedup vs baseline: 1.0373x; 1.0373x over previous
"""Causal self-attention (B=2, T=2048, C=1024, H=16) on 8 TRN2 NeuronCores.

Sharding: core = (batch b, head-group hg) with b in {0,1}, hg in {0..3};
each core computes Q/K/V projections and attention for its 4 heads on its
batch, plus the row-parallel slice of the output projection. The host sums
the 4 per-core partial projections per batch (bf16) and adds the output bias.

Device algorithm (all layouts transposed so softmax needs no on-chip
transposes):
  - x and weights are pre-gathered HOST-side into the exact SBUF layouts so
    every DMA line is >=1KB contiguous (~340 GB/s instead of ~210); tg=0's x
    and the pair-0 weights are split in half so the first chains start after
    ~0.3 MB; a burst of tiny warm-up matmuls on a memset tile (no DMA dep)
    releases the HAM clock throttle before the first real chain.
  - Q^T, K^T [dd, t] and V [t, dd] via bf16 matmul chains (contraction over C).
  - S^T[s, t]: 2 heads row-packed in the PE array (K=64 at row offsets 0/64).
  - exp on ScalarE straight out of PSUM (scale=1/sqrt(d) folded in); causal
    masking = one additive 128x128 band on diagonal blocks + trimming the
    S matmul, the exp AND the AV matmul to the unmasked column range;
    softmax denominators from an all-ones column appended to V (M=65
    matmul); normalization deferred to after AV. The tail block's
    reciprocal comes from a K=1 all-ones broadcast matmul + DVE reciprocal
    in PSUM (no DRAM bounce).
  - softmax reciprocals: denominator row bounced through DRAM into a
    [128, 8] partition-major tile, one DVE reciprocal op (ScalarE runs
    exps only), bounced back and broadcast-read for the normalize muls.
  - Emission interleaves projection/QKV chains into the attention blocks'
    spare PE slots (attention is ScalarE-exp paced) so the PE never idles.
  - y_partial[t, e] bf16 out; host sums partials in f32.
"""

import math
from collections import deque
from functools import lru_cache

import ml_dtypes
import numpy as np

import concourse.bass as bass
import concourse.mybir as mybir
from concourse import bacc
import concourse.tile as tile
from concourse import bass_utils

F32 = mybir.dt.float32
BF16 = mybir.dt.bfloat16
EXP = mybir.ActivationFunctionType.Exp

B, T, C, H = 2, 2048, 1024, 16
NCORES = 8
NH = 4            # heads per core
D = C // H        # 64
DD = NH * D       # 256 channels per core
P = 128
TG = 512          # t-group width (matmul moving dim)
NG = T // TG      # 4
NT = T // P       # 16 s-chunks
CCH = C // P      # 8 contraction chunks
NEG = -8.0e6      # pre-scale additive mask; *0.125 = -1e6 like the reference

LAST_RESULTS = None  # BassKernelResults of the most recent run (for test.py)


class FillerQueue:
    """PE work units interleaved into the attention blocks' spare slots."""

    def __init__(self):
        self.q = deque()

    def add(self, units):
        self.q.extend(units)

    def add_front(self, units):
        for u in reversed(units):
            self.q.appendleft(u)

    def pump(self, n=1):
        for _ in range(n):
            if not self.q:
                return
            self.q.popleft()()

    def flush(self):
        while self.q:
            self.q.popleft()()


def build_program(apply_kbias: bool, general_mask: bool) -> bass.Bass:
    nc = bacc.Bacc("TRN2", target_bir_lowering=False, debug=False,
                   enable_asserts=False)

    # host pre-gathered layouts (contiguous 1-8 KB DMA lines; the on-device
    # strided gathers ran the input stream at ~210 GB/s instead of ~340)
    xg = nc.dram_tensor("xg", [NG * P, CCH * TG], BF16, kind="ExternalInput").ap()
    wqg = nc.dram_tensor("wqg", [2 * P, CCH * P], BF16, kind="ExternalInput").ap()
    wkg = nc.dram_tensor("wkg", [2 * P, CCH * P], BF16, kind="ExternalInput").ap()
    wvg = nc.dram_tensor("wvg", [P, CCH * DD], BF16, kind="ExternalInput").ap()
    wpT = nc.dram_tensor("wpT", [DD, C], BF16, kind="ExternalInput").ap()
    bqk = nc.dram_tensor("bqk", [P, 4], F32, kind="ExternalInput").ap()
    bv_in = nc.dram_tensor("bv_sb", [P, DD], F32, kind="ExternalInput").ap()
    ident_in = nc.dram_tensor("ident", [D, D], BF16, kind="ExternalInput").ap()
    kbias_in = None
    if apply_kbias:
        kbias_in = nc.dram_tensor("kbias", [P, NT], F32, kind="ExternalInput").ap()
    band_in = maskT = None
    if general_mask:
        maskT = nc.dram_tensor("maskT", [T, T], F32, kind="ExternalInput").ap()
    else:
        band_in = nc.dram_tensor("band", [P, P], F32, kind="ExternalInput").ap()
    yp = nc.dram_tensor("yp", [T, C], BF16, kind="ExternalOutput").ap()
    # DRAM bounce buffers for the softmax denominators: raw rows land in
    # rcd_raw, get re-read [128, 8] partition-major (contiguous 8-elem lines),
    # reciprocated on DVE, written back t-major to rcd_rcp, then broadcast
    # across partitions (DMA from DRAM may use a 0-step partition dim).
    rcd_raw = nc.dram_tensor("rcd_raw", [2 * NG, 2 * TG], BF16, kind="Internal").ap()
    rcd_rcp = nc.dram_tensor("rcd_rcp", [2 * NG, 2 * TG], BF16, kind="Internal").ap()

    with tile.TileContext(nc) as tc:
        with tc.tile_pool(name="wts", bufs=1) as wts, \
             tc.tile_pool(name="xtp", bufs=1) as xtp, \
             tc.tile_pool(name="qkv", bufs=1) as qkv, \
             tc.tile_pool(name="otp", bufs=1) as otp, \
             tc.tile_pool(name="ptp", bufs=4) as ptp, \
             tc.tile_pool(name="asb", bufs=4) as asbp, \
             tc.tile_pool(name="rtp", bufs=2) as rtp, \
             tc.tile_pool(name="bcp", bufs=2) as bcp, \
             tc.tile_pool(name="tmp", bufs=3) as tmpp, \
             tc.tile_pool(name="ysb", bufs=6) as ysbp, \
             tc.tile_pool(name="mkp", bufs=2) as mkp, \
             tc.tile_pool(name="stp", bufs=2, space="PSUM") as stp, \
             tc.tile_pool(name="avp", bufs=2, space="PSUM") as avp, \
             tc.tile_pool(name="mmp", bufs=2, space="PSUM") as mmp:

            # Only Exp (and friends) are needed; preload so the act-table
            # DMA overlaps the input DMAs instead of stalling the first exp.
            from concourse.hw_specs import get_activation_tables
            tables = get_activation_tables(nc.m.arch)
            set_id = list(tables).index("natural_log_exp_and_others")
            nc.scalar.add_instruction(mybir.InstLoadActFuncSet(
                name=nc.get_next_instruction_name(), ins=[], outs=[],
                act_func_set_id=set_id))

            # ---- all-ones tile: warm-up stationary operand (no DMA dep so
            # the warm-up starts right after the preamble) + the K=1
            # broadcast matmul of the tail block's softmax denominators.
            ones_t = wts.tile([P, D], BF16, name="ones_t")
            nc.gpsimd.memset(ones_t, 1.0)

            # ---- input DMAs ----
            # gpsimd queue: x (t-group-major, first half of tg=0 leads) +
            # the small f32 tables; sync queue: weights, Q before K (the
            # chains run Q first) with each pair-0 weight split in half so
            # the first chain units start after ~0.25 MB.
            bqk_t = wts.tile([P, 4], F32, name="bqk_t")

            wkh = [wts.tile([P, CCH * P], BF16, name=f"wkh{i}") for i in range(2)]
            wqh = [wts.tile([P, CCH * P], BF16, name=f"wqh{i}") for i in range(2)]
            wvall = wts.tile([P, CCH * DD], BF16, name="wvall")
            xt0 = [xtp.tile([P, 4 * TG], BF16, name=f"xt0{h}") for h in range(2)]
            xtg = [None] + [xtp.tile([P, CCH * TG], BF16, name=f"xtg{t_}")
                            for t_ in range(1, NG)]

            # Per-queue DMA rings stream sequentially at ~200-340 GB/s, so
            # critical-path bytes are split across the three DMA-capable
            # queues and the not-yet-needed x t-groups go BEHIND the pair-1
            # weights: gpsimd carries only tg=0's x, scalar the V weights +
            # tables, sync the Q/K weights then the x tail.
            nc.gpsimd.dma_start(out=xt0[0], in_=xg[0:P, 0:4 * TG])
            nc.gpsimd.dma_start(out=xt0[1], in_=xg[0:P, 4 * TG:8 * TG])

            nc.sync.dma_start(out=wqh[0], in_=wqg[0:P, :])
            nc.sync.dma_start(out=wkh[0], in_=wkg[0:P, :])
            nc.sync.dma_start(out=bqk_t, in_=bqk)
            if kbias_in is not None:
                kbias_t = wts.tile([P, NT], F32, name="kbias_t")
                nc.sync.dma_start(out=kbias_t, in_=kbias_in)
            nc.sync.dma_start(out=xtg[1], in_=xg[P:2 * P, :])
            nc.sync.dma_start(out=wqh[1], in_=wqg[P:2 * P, :])
            nc.sync.dma_start(out=wkh[1], in_=wkg[P:2 * P, :])
            nc.sync.dma_start(out=xtg[2], in_=xg[2 * P:3 * P, :])
            nc.sync.dma_start(out=xtg[3], in_=xg[3 * P:4 * P, :])

            bv_sb = wts.tile([P, DD], F32, name="bv_t")
            band_t = None
            if band_in is not None:
                band_t = wts.tile([P, P], F32, name="band_t")
                nc.scalar.dma_start(out=band_t, in_=band_in)
            ident_t = wts.tile([D, D], BF16, name="ident_t")
            nc.scalar.dma_start(out=ident_t, in_=ident_in)
            nc.scalar.dma_start(out=bv_sb, in_=bv_in)
            nc.scalar.dma_start(out=wvall, in_=wvg)
            wp = [wts.tile([P, C], BF16, name=f"wp{i}") for i in range(2)]
            for i in range(2):
                nc.scalar.dma_start(out=wp[i], in_=wpT[i * P:(i + 1) * P, :])

            qt = [qkv.tile([P, T], BF16, name=f"qt{i}") for i in range(2)]
            kt = [qkv.tile([P, T], BF16, name=f"kt{i}") for i in range(2)]
            vaug = [qkv.tile([P, NH * (D + 1)], BF16, name=f"vaug{j}")
                    for j in range(NT)]
            ot = [otp.tile([P, T], BF16, name=f"ot{i}") for i in range(2)]

            # ---- HAM warm-up: ~3us of tiny matmuls so the PE clock is at
            # 8/8 by the time the first projection chain lands.
            # N=64 so 96 matmuls span ~5us of continuous PE activity (the
            # HAM SHORT window needs >=3.4us busy to unthrottle the clock)
            wps = mmp.tile([P, TG], F32, name="mm", tag="mm")
            for _ in range(96):
                nc.tensor.matmul(wps[0:1, 0:D], lhsT=ones_t[:, 0:1],
                                 rhs=ones_t[:, 0:D], start=True, stop=True)

            # ---- QKV chain units ----
            def wsl(whalves, c, iw):
                return whalves[iw][:, c * P:(c + 1) * P]

            def xsl(tg, c):
                if tg == 0:
                    return xt0[c // 4][:, (c % 4) * TG:(c % 4 + 1) * TG]
                return xtg[tg][:, c * TG:(c + 1) * TG]

            def xvsl(j, c):
                tg, u = divmod(j, NG)
                base = xsl(tg, c)
                return base[:, u * P:u * P + P]

            def qk_chain_units(iw, tg):
                """K then Q projection chain for dd-tile iw, t-group tg.
                Split into 2-matmul units + a trailing bias/drain unit (the
                drain lags its chain by one pump slot to avoid head-of-line
                blocking on the DVE queue)."""
                units = []
                for wall, dst, bcol in ((wqh, qt, iw), (wkh, kt, 2 + iw)):
                    box = {}

                    def mk_mm(c0, wall=wall, box=box):
                        def f():
                            if c0 == 0:
                                box['ps'] = mmp.tile([P, TG], F32, name="mm",
                                                     tag="mm")
                            for c in (c0, c0 + 1):
                                nc.tensor.matmul(
                                    box['ps'], lhsT=wsl(wall, c, iw),
                                    rhs=xsl(tg, c),
                                    start=(c == 0), stop=(c == CCH - 1))
                        return f

                    def mk_bias(dst=dst, bcol=bcol, box=box):
                        def f():
                            nc.vector.tensor_scalar_add(
                                dst[iw][:, tg * TG:(tg + 1) * TG], box['ps'],
                                bqk_t[:, bcol:bcol + 1])
                        return f

                    units += [mk_mm(0), mk_mm(2), mk_mm(4), mk_mm(6),
                              mk_bias()]
                return units

            def v_chain_units(j):
                box = {}

                def mk_mm(c0):
                    def f():
                        if c0 == 0:
                            box['ps'] = mmp.tile([P, TG], F32, name="mm",
                                                 tag="mm")
                        for c in range(c0, c0 + 4):
                            nc.tensor.matmul(
                                box['ps'][:, :DD], lhsT=xvsl(j, c),
                                rhs=wvall[:, c * DD:(c + 1) * DD],
                                start=(c == 0), stop=(c == CCH - 1))
                    return f

                def drain():
                    ps = box['ps']
                    vview = vaug[j].rearrange("p (h x) -> p h x", h=NH)
                    bvv = bv_sb.rearrange("p (h x) -> p h x", h=NH)
                    # ones column (softmax denominator row): in0*0 + 1
                    nc.vector.tensor_scalar(
                        vview[:, :, D:D + 1], bvv[:, :, 0:1], 0.0, 1.0,
                        mybir.AluOpType.mult, mybir.AluOpType.add)
                    nc.vector.tensor_add(
                        vview[:, :, 0:D],
                        ps[:, :DD].rearrange("p (h x) -> p h x", h=NH), bvv)

                return [mk_mm(0), mk_mm(4), drain]

            def proj_units(tt, ec):
                box = {}
                alt = (tt * 2 + ec) % 2

                def mm():
                    box['ps'] = mmp.tile([P, TG], F32, name="mm", tag="mm")
                    for i2 in range(2):
                        nc.tensor.matmul(
                            box['ps'], lhsT=ot[i2][:, tt * P:(tt + 1) * P],
                            rhs=wp[i2][:, ec * TG:(ec + 1) * TG],
                            start=(i2 == 0), stop=(i2 == 1))

                def drain():
                    # alternate the PSUM->SBUF cast and the store DMA across
                    # engines/queues so the drains pipeline 2-wide
                    ysb = ysbp.tile([P, TG], BF16, name="ysb", tag="ysb")
                    if alt:
                        nc.scalar.activation(
                            ysb, box['ps'], mybir.ActivationFunctionType.Copy)
                        nc.gpsimd.dma_start(
                            out=yp[tt * P:(tt + 1) * P,
                                   ec * TG:(ec + 1) * TG], in_=ysb)
                    else:
                        nc.vector.tensor_copy(ysb, box['ps'])
                        nc.sync.dma_start(
                            out=yp[tt * P:(tt + 1) * P,
                                   ec * TG:(ec + 1) * TG], in_=ysb)

                return [mm, drain]

            # ---- attention ----
            def attn_block(i, g, fq, pending=None, defer_av=False,
                           fast_recip=False, late_fq=None, tail_units=None):
                # causal: only s-chunks on/below the diagonal contribute.
                # `pending` is the previous block's deferred epilogue tail,
                # emitted after S(1) so its DMA-bounce waits never block this
                # block's DVE stream. `defer_av` emits all S's before any AV
                # (first block: the V chains feeding AV are still in fq).
                nj = NT if general_mask else 4 * g + 4
                av = [avp.tile([P, TG], F32, name="av", tag="av")
                      for _ in range(2)]
                pump_n = 2 if nj == 8 else 3

                def pump(n):
                    for _ in range(n):
                        if late_fq is not None and late_fq.q:
                            late_fq.q.popleft()()
                        elif fq.q:
                            fq.q.popleft()()

                def emit_S(j):
                    r = j - 4 * g
                    # causal trim: s-chunk j only attends to t >= r*P within
                    # this t-group; skip the masked-out left columns in the
                    # S matmul AND the exp (the AV matmul already trims the
                    # same range).
                    trim = r * P if (r >= 1 and not general_mask) else 0
                    st = stp.tile([P, 2 * TG], F32, name="st", tag="st")
                    for h in range(2):
                        nc.tensor.matmul(
                            st[:, h * TG + trim:(h + 1) * TG],
                            lhsT=(kt[i][64 * h:64 * h + 64,
                                        j * P:(j + 1) * P]),
                            rhs=(qt[i][64 * h:64 * h + 64,
                                       g * TG + trim:(g + 1) * TG]),
                            start=True, stop=True,
                            tile_position=(64 * h, 0))
                    if general_mask:
                        mk = mkp.tile([P, TG], F32, name="mk", tag="mk")
                        nc.sync.dma_start(
                            out=mk,
                            in_=maskT[j * P:(j + 1) * P, g * TG:(g + 1) * TG])
                        for h in range(2):
                            nc.vector.tensor_add(
                                st[:, h * TG:(h + 1) * TG],
                                st[:, h * TG:(h + 1) * TG], mk)
                    elif r >= 0:
                        for h in range(2):
                            sl = slice(h * TG + r * P, h * TG + (r + 1) * P)
                            nc.vector.tensor_add(st[:, sl], st[:, sl], band_t)
                    if apply_kbias:
                        for h in range(2):
                            nc.vector.tensor_scalar_add(
                                st[:, h * TG + trim:(h + 1) * TG],
                                st[:, h * TG + trim:(h + 1) * TG],
                                kbias_t[:, j:j + 1])
                    pt = ptp.tile([P, 2 * TG], BF16, name="pt", tag="pt")
                    if trim:
                        stv = st.rearrange("p (h t) -> p h t", h=2)
                        ptv = pt.rearrange("p (h t) -> p h t", h=2)
                        nc.scalar.activation(ptv[:, :, trim:], stv[:, :, trim:],
                                             EXP, scale=1.0 / math.sqrt(D))
                    else:
                        nc.scalar.activation(pt, st, EXP,
                                             scale=1.0 / math.sqrt(D))
                    return pt

                def emit_AV(j, pt):
                    r = j - 4 * g
                    trim = r * P if (r > 0 and not general_mask) else 0
                    for h in range(2):
                        nc.tensor.matmul(
                            av[h][0:D + 1, trim:TG],
                            lhsT=(vaug[j][:, (2 * i + h) * (D + 1):
                                          (2 * i + h + 1) * (D + 1)]),
                            rhs=(pt[:, h * TG + trim:(h + 1) * TG]),
                            start=(j == 0), stop=(j == nj - 1),
                            skip_group_check=True)

                prev = None
                for j in range(nj):
                    pt = emit_S(j)
                    if j == 1 and pending is not None:
                        pending()
                        pending = None
                    if prev is not None and not defer_av:
                        if late_fq is not None and prev[0] == nj - 4:
                            late_fq.flush()
                        emit_AV(*prev)
                    if not defer_av:
                        prev = (j, pt)
                    else:
                        prev = prev or []
                        prev.append((j, pt))
                    if j >= 1:
                        pump(pump_n)
                if defer_av:
                    fq.flush()
                    for j, pt in prev:
                        emit_AV(j, pt)
                else:
                    if late_fq is not None and prev[0] == nj - 4:
                        late_fq.flush()
                    emit_AV(*prev)

                # Epilogue part A: free the accumulator banks, launch the
                # denominator row into the DRAM transpose bounce.
                slot = i * NG + g
                asb = asbp.tile([D + 1, 2 * TG], BF16, name="asb", tag="asb")
                nc.vector.tensor_copy(asb[:, 0:TG], av[0][0:D + 1, :])
                nc.vector.tensor_copy(asb[:, TG:2 * TG], av[1][0:D + 1, :])

                def normalize(bc):
                    # upper-half (tm) first: its extra DMA hop into ot is on
                    # the critical path of the following projection
                    tm = tmpp.tile([P, TG], BF16, name="tm", tag="tm")
                    nc.vector.tensor_mul(tm[0:D, :], asb[0:D, TG:2 * TG],
                                         bc[0:D, TG:2 * TG])
                    nc.sync.dma_start(
                        out=ot[i][D:P, g * TG:(g + 1) * TG],
                        in_=tm[0:D, :])
                    nc.vector.tensor_mul(
                        ot[i][0:D, g * TG:(g + 1) * TG],
                        asb[0:D, 0:TG], bc[0:D, 0:TG])

                if fast_recip:
                    # Tail block. The softmax denominator row (asb row D,
                    # already in SBUF from the bank-freeing copies) is
                    # broadcast across partitions 0..63 with a K=1 all-ones
                    # matmul into a free PSUM pair, reciprocated on the DVE -
                    # no DRAM bounce, ~6us less dead latency than the old
                    # ln/exp + double-DMA path. Meanwhile the final
                    # projection's accumulation chains OPEN with their
                    # ot[0]-half (ready since phase 1) across the free PSUM
                    # banks, and CLOSE with the ot[1]-half after normalize.
                    units = [(t_, e_) for t_ in range(4 * g, 4 * g + 4)
                             for e_ in range(2)]
                    tp = []
                    st_t = stp.tile([P, 2 * TG], F32, name="st", tag="st")

                    def open_unit(bi, tt, ec):
                        if bi < 2:
                            ps = st_t[:, bi * TG:(bi + 1) * TG]
                        elif bi < 4:
                            ps = avp.tile([P, TG], F32, name="av", tag="av")
                        else:
                            ps = mmp.tile([P, TG], F32, name="mm", tag="mm")
                        nc.tensor.matmul(
                            ps, lhsT=ot[0][:, tt * P:(tt + 1) * P],
                            rhs=wp[0][:, ec * TG:(ec + 1) * TG],
                            start=True, stop=False, skip_group_check=True)
                        tp.append((tt, ec, ps))

                    for bi, (tt, ec) in enumerate(units[:4]):
                        open_unit(bi, tt, ec)
                    # denominator broadcast across partitions (K=1 matmul on
                    # the all-ones column) + fast DVE reciprocal from PSUM
                    bcps = stp.tile([P, 2 * TG], F32, name="st", tag="st")
                    for h in range(2):
                        nc.tensor.matmul(
                            bcps[0:D, h * TG:(h + 1) * TG],
                            lhsT=ones_t[D:D + 1, 0:D],
                            rhs=asb[D:D + 1, h * TG:(h + 1) * TG],
                            start=True, stop=True)
                    # 2 more opens issued behind the broadcast keep the PE
                    # warm through the reciprocal+normalize window
                    for bi in (4, 5):
                        open_unit(bi, *units[bi])
                    bc = bcp.tile([P, 2 * TG], F32, name="bcf", tag="bcf")
                    # ~51-ULP Newton-Raphson reciprocal: 5x faster than the
                    # exact DVE reciprocal (which measured 6.5us on [64,1024])
                    nc.vector.reciprocal_approx_fast(bc[0:D, :], bcps[0:D, :])
                    # normalize; the heads-2/3 half moves to partitions 64..127
                    # via an identity matmul into bcps' upper partitions + a
                    # DVE copy instead of the ~2.5us SBUF->SBUF DMA round trip
                    tm = tmpp.tile([P, TG], BF16, name="tm", tag="tm")
                    nc.vector.tensor_mul(tm[0:D, :], asb[0:D, TG:2 * TG],
                                         bc[0:D, TG:2 * TG])
                    nc.tensor.matmul(
                        bcps[D:P, 0:TG], lhsT=ident_t, rhs=tm[0:D, :],
                        start=True, stop=True)
                    nc.vector.tensor_copy(ot[i][D:P, g * TG:(g + 1) * TG],
                                          bcps[D:P, 0:TG])
                    nc.vector.tensor_mul(
                        ot[i][0:D, g * TG:(g + 1) * TG],
                        asb[0:D, 0:TG], bc[0:D, 0:TG])

                    def pdrain(bi, tt, ec, ps):
                        # casts alternate scalar/vector; stores all on the
                        # sync ring (the gpsimd ring's flush was the last
                        # thing to finish in the teardown)
                        ysb = ysbp.tile([P, TG], BF16, name="ysb", tag="ysb")
                        if bi % 2:
                            nc.scalar.activation(
                                ysb, ps, mybir.ActivationFunctionType.Copy)
                        else:
                            nc.vector.tensor_copy(ysb, ps)
                        nc.sync.dma_start(
                            out=yp[tt * P:(tt + 1) * P,
                                   ec * TG:(ec + 1) * TG], in_=ysb)

                    for bi, (tt, ec, ps) in enumerate(tp):
                        nc.tensor.matmul(
                            ps, lhsT=ot[1][:, tt * P:(tt + 1) * P],
                            rhs=wp[1][:, ec * TG:(ec + 1) * TG],
                            start=False, stop=True, skip_group_check=True)
                        pdrain(bi, tt, ec, ps)
                    # last 2 units had no free PSUM bank for an early open
                    # (bcps holds 2 banks); run them as plain pairs
                    for bi, (tt, ec) in enumerate(units[6:]):
                        ps = mmp.tile([P, TG], F32, name="mm", tag="mm")
                        for i2 in range(2):
                            nc.tensor.matmul(
                                ps, lhsT=ot[i2][:, tt * P:(tt + 1) * P],
                                rhs=wp[i2][:, ec * TG:(ec + 1) * TG],
                                start=(i2 == 0), stop=(i2 == 1))
                        pdrain(bi, tt, ec, ps)
                    return None

                nc.gpsimd.dma_start(out=rcd_raw[slot], in_=asb[D:D + 1, :])
                rt = rtp.tile([P, 8], BF16, name="rt", tag="rt")
                nc.gpsimd.dma_start(out=rt, in_=bass.AP(
                    tensor=rcd_raw.tensor, offset=rcd_raw[slot].offset,
                    ap=[[8, P], [1, 8]]))

                # Epilogue part B (deferred into the next block so the
                # bounce round-trips never stall this DVE/sync stream).
                def part_b():
                    rw = rtp.tile([P, 8], BF16, name="rw", tag="rw")
                    with nc.allow_low_precision(
                            reason="bf16 softmax denominators (~0.4% rel)"):
                        nc.vector.reciprocal(rw, rt)
                    nc.gpsimd.dma_start(out=bass.AP(
                        tensor=rcd_rcp.tensor, offset=rcd_rcp[slot].offset,
                        ap=[[8, P], [1, 8]]), in_=rw)
                    bc = bcp.tile([P, 2 * TG], BF16, name="bc", tag="bc")
                    nc.gpsimd.dma_start(out=bc[0:D, :], in_=bass.AP(
                        tensor=rcd_rcp.tensor, offset=rcd_rcp[slot].offset,
                        ap=[[0, D], [1, 2 * TG]]))
                    normalize(bc)

                return part_b

            # ---- emission schedule ----
            # Pair 0's first chains + V(0..3) run before its g=0 block; the
            # rest of QKV, pair-1 chains and the output projections are fed
            # through the filler queue into the attention blocks' spare PE
            # slots (attention is exp-paced on ScalarE).
            fq = FillerQueue()
            fq.add(qk_chain_units(0, 0))
            fq.flush()
            for j in range(NT if general_mask else 4):
                fq.add(v_chain_units(j))
            pend = None
            cur_late = None
            for g in range(NG):
                if g < NG - 1:
                    fq.add(qk_chain_units(0, g + 1))
                    nxt_late = None
                    if not general_mask:
                        nxt_late = FillerQueue()
                        for j in range(4 * (g + 1), 4 * (g + 2)):
                            nxt_late.add(v_chain_units(j))
                else:
                    fq.add(qk_chain_units(1, 0))
                    nxt_late = None
                pend = attn_block(0, g, fq, pending=pend, defer_av=(g == 0),
                                  late_fq=cur_late)
                fq.flush()
                cur_late = nxt_late
            for g in range(NG):
                if g < NG - 1:
                    fq.add_front(qk_chain_units(1, g + 1))
                pend = attn_block(1, g, fq, pending=pend,
                                  fast_recip=(g == NG - 1))
                fq.flush()
                if g < NG - 1:
                    for tt in range(4 * g, 4 * g + 4):
                        for ec in range(2):
                            fq.add(proj_units(tt, ec))
            if pend is not None:
                pend()
            fq.flush()

    nc.compile()
    return nc


@lru_cache(maxsize=4)
def _program(apply_kbias: bool, general_mask: bool) -> bass.Bass:
    return build_program(apply_kbias, general_mask)


def _host_prep(inputs):
    x = np.asarray(inputs["x"], np.float32)
    Wq = np.asarray(inputs["Wq"], np.float32)
    bq = np.asarray(inputs["bq"], np.float32)
    Wk = np.asarray(inputs["Wk"], np.float32)
    bk = np.asarray(inputs["bk"], np.float32)
    Wv = np.asarray(inputs["Wv"], np.float32)
    bv = np.asarray(inputs["bv"], np.float32)
    Wp = np.asarray(inputs["Wp"], np.float32)
    attn_mask = np.asarray(inputs["attn_mask"])
    valid = np.asarray(inputs["valid_input_mask"])

    tril = np.tril(np.ones((T, T), attn_mask.dtype))
    causal = all(np.array_equal(attn_mask[b], tril) for b in range(B))
    kbias_all = (valid.astype(np.float32) - 1.0) * 1e6  # [B, T]
    apply_kbias = bool((valid == 0).any())

    band = np.where(np.arange(P)[:, None] <= np.arange(P)[None, :],
                    np.float32(0.0), np.float32(NEG))

    # device-ready layouts (pre-gathered so every DMA line is >=1 KB
    # contiguous):
    #   xg[tg*P+p, c*TG+u]   = x[b][tg*TG+u, c*P+p]
    #   wqg[iw*P+p, c*P+m]   = Wq[sl][iw*P+m, c*P+p]   (same for wk)
    #   wvg[p, c*DD+q]       = Wv[sl][q, c*P+p]
    def xg_prep(xb):
        return np.ascontiguousarray(
            xb.reshape(NG, TG, CCH, P).transpose(0, 3, 2, 1)
            .reshape(NG * P, CCH * TG)).astype(ml_dtypes.bfloat16)

    def wqkg_prep(Wsl):
        return np.ascontiguousarray(
            Wsl.reshape(2, P, CCH, P).transpose(0, 3, 2, 1)
            .reshape(2 * P, CCH * P)).astype(ml_dtypes.bfloat16)

    def wvg_prep(Wsl):
        return np.ascontiguousarray(
            Wsl.reshape(DD, CCH, P).transpose(2, 1, 0)
            .reshape(P, CCH * DD)).astype(ml_dtypes.bfloat16)

    in_maps = []
    for core in range(NCORES):
        b, hg = divmod(core, 4)
        sl = slice(hg * DD, (hg + 1) * DD)
        m = {
            "xg": xg_prep(x[b]),
            "wqg": wqkg_prep(Wq[sl, :]),
            "wkg": wqkg_prep(Wk[sl, :]),
            "wvg": wvg_prep(Wv[sl, :]),
            "wpT": np.ascontiguousarray(Wp[:, sl].T).astype(ml_dtypes.bfloat16),
            "bqk": np.ascontiguousarray(
                np.stack([bq[sl][:P], bq[sl][P:], bk[sl][:P], bk[sl][P:]], 1)),
            "bv_sb": np.ascontiguousarray(np.tile(bv[sl], (P, 1))),
            "ident": np.eye(D, dtype=ml_dtypes.bfloat16),
        }
        if apply_kbias:
            m["kbias"] = np.ascontiguousarray(kbias_all[b].reshape(NT, P).T)
        if not causal:
            m["maskT"] = np.ascontiguousarray(
                (attn_mask[b].T.astype(np.float32) - 1.0) * (-NEG))
        else:
            m["band"] = band
        in_maps.append(m)
    return in_maps, apply_kbias, causal


def _run(inputs, trace=False, trace_cores=None):
    global LAST_RESULTS
    in_maps, apply_kbias, causal = _host_prep(inputs)
    nc = _program(apply_kbias, not causal)
    res = bass_utils.run_bass_kernel_spmd(
        nc, in_maps, core_ids=list(range(NCORES)), trace=trace,
        trace_cores=trace_cores)
    LAST_RESULTS = res

    bp = np.asarray(inputs["bp"], np.float32)
    y = np.zeros((B, T, C), np.float32)
    for core in range(NCORES):
        y[core // 4] += np.asarray(res.results[core]["yp"], np.float32)
    y += bp[None, None, :]
    return y


def kernel(**inputs) -> np.ndarray:
    return _run(inputs)



# revision 26
# speedup vs baseline: 1.0432x; 1.0057x over previous
"""Causal self-attention (B=2, T=2048, C=1024, H=16) on 8 TRN2 NeuronCores.

Sharding: core = (batch b, head-group hg) with b in {0,1}, hg in {0..3};
each core computes Q/K/V projections and attention for its 4 heads on its
batch, plus the row-parallel slice of the output projection. The host sums
the 4 per-core partial projections per batch (bf16) and adds the output bias.

Device algorithm (all layouts transposed so softmax needs no on-chip
transposes):
  - x and weights are pre-gathered HOST-side into the exact SBUF layouts so
    every DMA line is >=1KB contiguous (~340 GB/s instead of ~210); tg=0's x
    and the pair-0 weights are split in half so the first chains start after
    ~0.3 MB; a burst of tiny warm-up matmuls on a memset tile (no DMA dep)
    releases the HAM clock throttle before the first real chain.
  - Q^T, K^T [dd, t] and V [t, dd] via bf16 matmul chains (contraction over C).
  - S^T[s, t]: 2 heads row-packed in the PE array (K=64 at row offsets 0/64).
  - exp on ScalarE straight out of PSUM (scale=1/sqrt(d) folded in); causal
    masking = one additive 128x128 band on diagonal blocks + trimming the
    S matmul, the exp AND the AV matmul to the unmasked column range;
    softmax denominators from an all-ones column appended to V (M=65
    matmul); normalization deferred to after AV. The tail block's
    reciprocal comes from a K=1 all-ones broadcast matmul + DVE reciprocal
    in PSUM (no DRAM bounce).
  - softmax reciprocals: denominator row bounced through DRAM into a
    [128, 8] partition-major tile, one DVE reciprocal op (ScalarE runs
    exps only), bounced back and broadcast-read for the normalize muls.
  - Emission interleaves projection/QKV chains into the attention blocks'
    spare PE slots (attention is ScalarE-exp paced) so the PE never idles.
  - y_partial[t, e] bf16 out; host sums partials in f32.
"""

import math
from collections import deque
from functools import lru_cache

import ml_dtypes
import numpy as np

import concourse.bass as bass
import concourse.mybir as mybir
from concourse import bacc
import concourse.tile as tile
from concourse import bass_utils

F32 = mybir.dt.float32
BF16 = mybir.dt.bfloat16
EXP = mybir.ActivationFunctionType.Exp

B, T, C, H = 2, 2048, 1024, 16
NCORES = 8
NH = 4            # heads per core
D = C // H        # 64
DD = NH * D       # 256 channels per core
P = 128
TG = 512          # t-group width (matmul moving dim)
NG = T // TG      # 4
NT = T // P       # 16 s-chunks
CCH = C // P      # 8 contraction chunks
NEG = -8.0e6      # pre-scale additive mask; *0.125 = -1e6 like the reference

LAST_RESULTS = None  # BassKernelResults of the most recent run (for test.py)


class FillerQueue:
    """PE work units interleaved into the attention blocks' spare slots."""

    def __init__(self):
        self.q = deque()

    def add(self, units):
        self.q.extend(units)

    def add_front(self, units):
        for u in reversed(units):
            self.q.appendleft(u)

    def pump(self, n=1):
        for _ in range(n):
            if not self.q:
                return
            self.q.popleft()()

    def flush(self):
        while self.q:
            self.q.popleft()()


def build_program(apply_kbias: bool, general_mask: bool) -> bass.Bass:
    nc = bacc.Bacc("TRN2", target_bir_lowering=False, debug=False,
                   enable_asserts=False)

    # host pre-gathered layouts (contiguous 1-8 KB DMA lines; the on-device
    # strided gathers ran the input stream at ~210 GB/s instead of ~340)
    xg = nc.dram_tensor("xg", [NG * P, CCH * TG], BF16, kind="ExternalInput").ap()
    wqg = nc.dram_tensor("wqg", [2 * P, CCH * P], BF16, kind="ExternalInput").ap()
    wkg = nc.dram_tensor("wkg", [2 * P, CCH * P], BF16, kind="ExternalInput").ap()
    wvg = nc.dram_tensor("wvg", [P, CCH * DD], BF16, kind="ExternalInput").ap()
    wpT = nc.dram_tensor("wpT", [DD, C], BF16, kind="ExternalInput").ap()
    bqk = nc.dram_tensor("bqk", [P, 4], F32, kind="ExternalInput").ap()
    bv_in = nc.dram_tensor("bv_sb", [P, DD], F32, kind="ExternalInput").ap()
    ident_in = nc.dram_tensor("ident", [D, D], BF16, kind="ExternalInput").ap()
    kbias_in = None
    if apply_kbias:
        kbias_in = nc.dram_tensor("kbias", [P, NT], F32, kind="ExternalInput").ap()
    band_in = maskT = None
    if general_mask:
        maskT = nc.dram_tensor("maskT", [T, T], F32, kind="ExternalInput").ap()
    else:
        band_in = nc.dram_tensor("band", [P, P], F32, kind="ExternalInput").ap()
    yp = nc.dram_tensor("yp", [T, C], BF16, kind="ExternalOutput").ap()
    # DRAM bounce buffers for the softmax denominators: raw rows land in
    # rcd_raw, get re-read [128, 8] partition-major (contiguous 8-elem lines),
    # reciprocated on DVE, written back t-major to rcd_rcp, then broadcast
    # across partitions (DMA from DRAM may use a 0-step partition dim).
    rcd_raw = nc.dram_tensor("rcd_raw", [2 * NG, 2 * TG], BF16, kind="Internal").ap()
    rcd_rcp = nc.dram_tensor("rcd_rcp", [2 * NG, 2 * TG], BF16, kind="Internal").ap()

    with tile.TileContext(nc) as tc:
        with tc.tile_pool(name="wts", bufs=1) as wts, \
             tc.tile_pool(name="xtp", bufs=1) as xtp, \
             tc.tile_pool(name="qkv", bufs=1) as qkv, \
             tc.tile_pool(name="otp", bufs=1) as otp, \
             tc.tile_pool(name="ptp", bufs=4) as ptp, \
             tc.tile_pool(name="asb", bufs=4) as asbp, \
             tc.tile_pool(name="rtp", bufs=2) as rtp, \
             tc.tile_pool(name="bcp", bufs=2) as bcp, \
             tc.tile_pool(name="tmp", bufs=3) as tmpp, \
             tc.tile_pool(name="ysb", bufs=6) as ysbp, \
             tc.tile_pool(name="mkp", bufs=2) as mkp, \
             tc.tile_pool(name="stp", bufs=2, space="PSUM") as stp, \
             tc.tile_pool(name="avp", bufs=2, space="PSUM") as avp, \
             tc.tile_pool(name="mmp", bufs=2, space="PSUM") as mmp:

            # Only Exp (and friends) are needed; preload so the act-table
            # DMA overlaps the input DMAs instead of stalling the first exp.
            from concourse.hw_specs import get_activation_tables
            tables = get_activation_tables(nc.m.arch)
            set_id = list(tables).index("natural_log_exp_and_others")
            nc.scalar.add_instruction(mybir.InstLoadActFuncSet(
                name=nc.get_next_instruction_name(), ins=[], outs=[],
                act_func_set_id=set_id))

            # ---- all-ones tile: warm-up stationary operand (no DMA dep so
            # the warm-up starts right after the preamble) + the K=1
            # broadcast matmul of the tail block's softmax denominators.
            ones_t = wts.tile([P, D], BF16, name="ones_t")
            nc.gpsimd.memset(ones_t, 1.0)

            # ---- input DMAs ----
            # gpsimd queue: x (t-group-major, first half of tg=0 leads) +
            # the small f32 tables; sync queue: weights, Q before K (the
            # chains run Q first) with each pair-0 weight split in half so
            # the first chain units start after ~0.25 MB.
            bqk_t = wts.tile([P, 4], F32, name="bqk_t")

            wkh = [wts.tile([P, CCH * P], BF16, name=f"wkh{i}") for i in range(2)]
            wqh = [wts.tile([P, CCH * P], BF16, name=f"wqh{i}") for i in range(2)]
            wvall = wts.tile([P, CCH * DD], BF16, name="wvall")
            xt0 = [xtp.tile([P, 4 * TG], BF16, name=f"xt0{h}") for h in range(2)]
            xtg = [None] + [xtp.tile([P, CCH * TG], BF16, name=f"xtg{t_}")
                            for t_ in range(1, NG)]

            # Per-queue DMA rings stream sequentially at ~200-340 GB/s, so
            # critical-path bytes are split across the three DMA-capable
            # queues and the not-yet-needed x t-groups go BEHIND the pair-1
            # weights: gpsimd carries only tg=0's x, scalar the V weights +
            # tables, sync the Q/K weights then the x tail.
            nc.gpsimd.dma_start(out=xt0[0], in_=xg[0:P, 0:4 * TG])
            nc.gpsimd.dma_start(out=xt0[1], in_=xg[0:P, 4 * TG:8 * TG])

            nc.sync.dma_start(out=wqh[0], in_=wqg[0:P, :])
            nc.sync.dma_start(out=wkh[0], in_=wkg[0:P, :])
            nc.sync.dma_start(out=bqk_t, in_=bqk)
            if kbias_in is not None:
                kbias_t = wts.tile([P, NT], F32, name="kbias_t")
                nc.sync.dma_start(out=kbias_t, in_=kbias_in)
            nc.sync.dma_start(out=xtg[1], in_=xg[P:2 * P, :])
            nc.sync.dma_start(out=wqh[1], in_=wqg[P:2 * P, :])
            nc.sync.dma_start(out=wkh[1], in_=wkg[P:2 * P, :])
            nc.sync.dma_start(out=xtg[2], in_=xg[2 * P:3 * P, :])
            nc.sync.dma_start(out=xtg[3], in_=xg[3 * P:4 * P, :])

            bv_sb = wts.tile([P, DD], F32, name="bv_t")
            band_t = None
            if band_in is not None:
                band_t = wts.tile([P, P], F32, name="band_t")
                nc.scalar.dma_start(out=band_t, in_=band_in)
            ident_t = wts.tile([D, D], BF16, name="ident_t")
            nc.scalar.dma_start(out=ident_t, in_=ident_in)
            nc.scalar.dma_start(out=bv_sb, in_=bv_in)
            nc.scalar.dma_start(out=wvall, in_=wvg)
            wp = [wts.tile([P, C], BF16, name=f"wp{i}") for i in range(2)]
            for i in range(2):
                nc.scalar.dma_start(out=wp[i], in_=wpT[i * P:(i + 1) * P, :])

            qt = [qkv.tile([P, T], BF16, name=f"qt{i}") for i in range(2)]
            kt = [qkv.tile([P, T], BF16, name=f"kt{i}") for i in range(2)]
            vaug = [qkv.tile([P, NH * (D + 1)], BF16, name=f"vaug{j}")
                    for j in range(NT)]
            ot = [otp.tile([P, T], BF16, name=f"ot{i}") for i in range(2)]

            # ---- HAM warm-up: ~3us of tiny matmuls so the PE clock is at
            # 8/8 by the time the first projection chain lands.
            # N=64 so 96 matmuls span ~5us of continuous PE activity (the
            # HAM SHORT window needs >=3.4us busy to unthrottle the clock)
            wps = mmp.tile([P, TG], F32, name="mm", tag="mm")
            for _ in range(96):
                nc.tensor.matmul(wps[0:1, 0:D], lhsT=ones_t[:, 0:1],
                                 rhs=ones_t[:, 0:D], start=True, stop=True)

            # ---- QKV chain units ----
            def wsl(whalves, c, iw):
                return whalves[iw][:, c * P:(c + 1) * P]

            def xsl(tg, c):
                if tg == 0:
                    return xt0[c // 4][:, (c % 4) * TG:(c % 4 + 1) * TG]
                return xtg[tg][:, c * TG:(c + 1) * TG]

            def xvsl(j, c):
                tg, u = divmod(j, NG)
                base = xsl(tg, c)
                return base[:, u * P:u * P + P]

            def qk_chain_units(iw, tg):
                """K then Q projection chain for dd-tile iw, t-group tg.
                Split into 2-matmul units + a trailing bias/drain unit (the
                drain lags its chain by one pump slot to avoid head-of-line
                blocking on the DVE queue)."""
                units = []
                for wall, dst, bcol in ((wqh, qt, iw), (wkh, kt, 2 + iw)):
                    box = {}

                    def mk_mm(c0, wall=wall, box=box):
                        def f():
                            if c0 == 0:
                                box['ps'] = mmp.tile([P, TG], F32, name="mm",
                                                     tag="mm")
                            for c in (c0, c0 + 1):
                                nc.tensor.matmul(
                                    box['ps'], lhsT=wsl(wall, c, iw),
                                    rhs=xsl(tg, c),
                                    start=(c == 0), stop=(c == CCH - 1))
                        return f

                    def mk_bias(dst=dst, bcol=bcol, box=box):
                        def f():
                            nc.vector.tensor_scalar_add(
                                dst[iw][:, tg * TG:(tg + 1) * TG], box['ps'],
                                bqk_t[:, bcol:bcol + 1])
                        return f

                    units += [mk_mm(0), mk_mm(2), mk_mm(4), mk_mm(6),
                              mk_bias()]
                return units

            def v_chain_units(j):
                box = {}

                def mk_mm(c0):
                    def f():
                        if c0 == 0:
                            box['ps'] = mmp.tile([P, TG], F32, name="mm",
                                                 tag="mm")
                        for c in range(c0, c0 + 4):
                            nc.tensor.matmul(
                                box['ps'][:, :DD], lhsT=xvsl(j, c),
                                rhs=wvall[:, c * DD:(c + 1) * DD],
                                start=(c == 0), stop=(c == CCH - 1))
                    return f

                def drain():
                    ps = box['ps']
                    vview = vaug[j].rearrange("p (h x) -> p h x", h=NH)
                    bvv = bv_sb.rearrange("p (h x) -> p h x", h=NH)
                    # ones column (softmax denominator row): in0*0 + 1
                    nc.vector.tensor_scalar(
                        vview[:, :, D:D + 1], bvv[:, :, 0:1], 0.0, 1.0,
                        mybir.AluOpType.mult, mybir.AluOpType.add)
                    nc.vector.tensor_add(
                        vview[:, :, 0:D],
                        ps[:, :DD].rearrange("p (h x) -> p h x", h=NH), bvv)

                return [mk_mm(0), mk_mm(4), drain]

            def proj_units(tt, ec):
                box = {}
                alt = (tt * 2 + ec) % 2

                def mm():
                    box['ps'] = mmp.tile([P, TG], F32, name="mm", tag="mm")
                    for i2 in range(2):
                        nc.tensor.matmul(
                            box['ps'], lhsT=ot[i2][:, tt * P:(tt + 1) * P],
                            rhs=wp[i2][:, ec * TG:(ec + 1) * TG],
                            start=(i2 == 0), stop=(i2 == 1))

                def drain():
                    # alternate the PSUM->SBUF cast and the store DMA across
                    # engines/queues so the drains pipeline 2-wide
                    ysb = ysbp.tile([P, TG], BF16, name="ysb", tag="ysb")
                    if alt:
                        nc.scalar.activation(
                            ysb, box['ps'], mybir.ActivationFunctionType.Copy)
                        nc.gpsimd.dma_start(
                            out=yp[tt * P:(tt + 1) * P,
                                   ec * TG:(ec + 1) * TG], in_=ysb)
                    else:
                        nc.vector.tensor_copy(ysb, box['ps'])
                        nc.sync.dma_start(
                            out=yp[tt * P:(tt + 1) * P,
                                   ec * TG:(ec + 1) * TG], in_=ysb)

                return [mm, drain]

            # ---- attention ----
            def attn_block(i, g, fq, pending=None, defer_av=False,
                           fast_recip=False, late_fq=None, tail_units=None):
                # causal: only s-chunks on/below the diagonal contribute.
                # `pending` is the previous block's deferred epilogue tail,
                # emitted after S(1) so its DMA-bounce waits never block this
                # block's DVE stream. `defer_av` emits all S's before any AV
                # (first block: the V chains feeding AV are still in fq).
                nj = NT if general_mask else 4 * g + 4
                av = [avp.tile([P, TG], F32, name="av", tag="av")
                      for _ in range(2)]
                pump_n = 2 if nj == 8 else 3

                def pump(n):
                    for _ in range(n):
                        if late_fq is not None and late_fq.q:
                            late_fq.q.popleft()()
                        elif fq.q:
                            fq.q.popleft()()

                def emit_S(j):
                    r = j - 4 * g
                    # causal trim: s-chunk j only attends to t >= r*P within
                    # this t-group; skip the masked-out left columns in the
                    # S matmul AND the exp (the AV matmul already trims the
                    # same range).
                    trim = r * P if (r >= 1 and not general_mask) else 0
                    st = stp.tile([P, 2 * TG], F32, name="st", tag="st")
                    if i == 1 or g == 3:
                        # keep-warm matmul: the exp-paced stretches of these
                        # blocks drop PE duty low enough that the HAM clock
                        # gate re-throttles to 1.2 GHz (~10us/run penalty).
                        # Writes [0:1, 0:64] of st bank 0, which the S h=0
                        # matmul (start=True) clears/overwrites right after.
                        nc.tensor.matmul(st[0:1, 0:D], lhsT=ones_t[:, 0:1],
                                         rhs=ones_t[:, 0:D], start=True,
                                         stop=True, skip_group_check=True)
                    for h in range(2):
                        nc.tensor.matmul(
                            st[:, h * TG + trim:(h + 1) * TG],
                            lhsT=(kt[i][64 * h:64 * h + 64,
                                        j * P:(j + 1) * P]),
                            rhs=(qt[i][64 * h:64 * h + 64,
                                       g * TG + trim:(g + 1) * TG]),
                            start=True, stop=True,
                            tile_position=(64 * h, 0))
                    if general_mask:
                        mk = mkp.tile([P, TG], F32, name="mk", tag="mk")
                        nc.sync.dma_start(
                            out=mk,
                            in_=maskT[j * P:(j + 1) * P, g * TG:(g + 1) * TG])
                        for h in range(2):
                            nc.vector.tensor_add(
                                st[:, h * TG:(h + 1) * TG],
                                st[:, h * TG:(h + 1) * TG], mk)
                    elif r >= 0:
                        for h in range(2):
                            sl = slice(h * TG + r * P, h * TG + (r + 1) * P)
                            nc.vector.tensor_add(st[:, sl], st[:, sl], band_t)
                    if apply_kbias:
                        for h in range(2):
                            nc.vector.tensor_scalar_add(
                                st[:, h * TG + trim:(h + 1) * TG],
                                st[:, h * TG + trim:(h + 1) * TG],
                                kbias_t[:, j:j + 1])
                    pt = ptp.tile([P, 2 * TG], BF16, name="pt", tag="pt")
                    if trim:
                        stv = st.rearrange("p (h t) -> p h t", h=2)
                        ptv = pt.rearrange("p (h t) -> p h t", h=2)
                        nc.scalar.activation(ptv[:, :, trim:], stv[:, :, trim:],
                                             EXP, scale=1.0 / math.sqrt(D))
                    else:
                        nc.scalar.activation(pt, st, EXP,
                                             scale=1.0 / math.sqrt(D))
                    return pt

                def emit_AV(j, pt):
                    r = j - 4 * g
                    trim = r * P if (r > 0 and not general_mask) else 0
                    for h in range(2):
                        nc.tensor.matmul(
                            av[h][0:D + 1, trim:TG],
                            lhsT=(vaug[j][:, (2 * i + h) * (D + 1):
                                          (2 * i + h + 1) * (D + 1)]),
                            rhs=(pt[:, h * TG + trim:(h + 1) * TG]),
                            start=(j == 0), stop=(j == nj - 1),
                            skip_group_check=True)

                prev = None
                for j in range(nj):
                    pt = emit_S(j)
                    if j == 1 and pending is not None:
                        pending()
                        pending = None
                    if prev is not None and not defer_av:
                        if late_fq is not None and prev[0] == nj - 4:
                            late_fq.flush()
                        emit_AV(*prev)
                    if not defer_av:
                        prev = (j, pt)
                    else:
                        prev = prev or []
                        prev.append((j, pt))
                    if j >= 1:
                        pump(pump_n)
                if defer_av:
                    fq.flush()
                    for j, pt in prev:
                        emit_AV(j, pt)
                else:
                    if late_fq is not None and prev[0] == nj - 4:
                        late_fq.flush()
                    emit_AV(*prev)

                # Epilogue part A: free the accumulator banks, launch the
                # denominator row into the DRAM transpose bounce.
                slot = i * NG + g
                asb = asbp.tile([D + 1, 2 * TG], BF16, name="asb", tag="asb")
                nc.vector.tensor_copy(asb[:, 0:TG], av[0][0:D + 1, :])
                nc.vector.tensor_copy(asb[:, TG:2 * TG], av[1][0:D + 1, :])

                def normalize(bc):
                    # upper-half (tm) first: its extra DMA hop into ot is on
                    # the critical path of the following projection
                    tm = tmpp.tile([P, TG], BF16, name="tm", tag="tm")
                    nc.vector.tensor_mul(tm[0:D, :], asb[0:D, TG:2 * TG],
                                         bc[0:D, TG:2 * TG])
                    nc.sync.dma_start(
                        out=ot[i][D:P, g * TG:(g + 1) * TG],
                        in_=tm[0:D, :])
                    nc.vector.tensor_mul(
                        ot[i][0:D, g * TG:(g + 1) * TG],
                        asb[0:D, 0:TG], bc[0:D, 0:TG])

                if fast_recip:
                    # Tail block. The softmax denominator row (asb row D,
                    # already in SBUF from the bank-freeing copies) is
                    # broadcast across partitions 0..63 with a K=1 all-ones
                    # matmul into a free PSUM pair, reciprocated on the DVE -
                    # no DRAM bounce, ~6us less dead latency than the old
                    # ln/exp + double-DMA path. Meanwhile the final
                    # projection's accumulation chains OPEN with their
                    # ot[0]-half (ready since phase 1) across the free PSUM
                    # banks, and CLOSE with the ot[1]-half after normalize.
                    units = [(t_, e_) for t_ in range(4 * g, 4 * g + 4)
                             for e_ in range(2)]
                    tp = []
                    st_t = stp.tile([P, 2 * TG], F32, name="st", tag="st")

                    def open_unit(bi, tt, ec):
                        if bi < 2:
                            ps = st_t[:, bi * TG:(bi + 1) * TG]
                        elif bi < 4:
                            ps = avp.tile([P, TG], F32, name="av", tag="av")
                        else:
                            ps = mmp.tile([P, TG], F32, name="mm", tag="mm")
                        nc.tensor.matmul(
                            ps, lhsT=ot[0][:, tt * P:(tt + 1) * P],
                            rhs=wp[0][:, ec * TG:(ec + 1) * TG],
                            start=True, stop=False, skip_group_check=True)
                        tp.append((tt, ec, ps))

                    for bi, (tt, ec) in enumerate(units[:4]):
                        open_unit(bi, tt, ec)
                    # denominator broadcast across partitions (K=1 matmul on
                    # the all-ones column) + fast DVE reciprocal from PSUM
                    bcps = stp.tile([P, 2 * TG], F32, name="st", tag="st")
                    # ~2us of keep-warm matmuls spanning the reciprocal +
                    # normalize window so the closes run at full clock; the
                    # h=0 broadcast (start=True) clears/overwrites the bank
                    for _ in range(10):
                        nc.tensor.matmul(bcps[0:1, 0:TG],
                                         lhsT=ones_t[:, 0:1],
                                         rhs=qt[1][0:P, 0:TG], start=True,
                                         stop=True, skip_group_check=True)
                    for h in range(2):
                        nc.tensor.matmul(
                            bcps[0:D, h * TG:(h + 1) * TG],
                            lhsT=ones_t[D:D + 1, 0:D],
                            rhs=asb[D:D + 1, h * TG:(h + 1) * TG],
                            start=True, stop=True)
                    # 2 more opens issued behind the broadcast keep the PE
                    # warm through the reciprocal+normalize window
                    for bi in (4, 5):
                        open_unit(bi, *units[bi])
                    bc = bcp.tile([P, 2 * TG], F32, name="bcf", tag="bcf")
                    # ~51-ULP Newton-Raphson reciprocal: 5x faster than the
                    # exact DVE reciprocal (which measured 6.5us on [64,1024])
                    nc.vector.reciprocal_approx_fast(bc[0:D, :], bcps[0:D, :])
                    # normalize; the heads-2/3 half moves to partitions 64..127
                    # via an identity matmul into bcps' upper partitions + a
                    # DVE copy instead of the ~2.5us SBUF->SBUF DMA round trip
                    tm = tmpp.tile([P, TG], BF16, name="tm", tag="tm")
                    nc.vector.tensor_mul(tm[0:D, :], asb[0:D, TG:2 * TG],
                                         bc[0:D, TG:2 * TG])
                    nc.tensor.matmul(
                        bcps[D:P, 0:TG], lhsT=ident_t, rhs=tm[0:D, :],
                        start=True, stop=True)
                    nc.vector.tensor_copy(ot[i][D:P, g * TG:(g + 1) * TG],
                                          bcps[D:P, 0:TG])
                    nc.vector.tensor_mul(
                        ot[i][0:D, g * TG:(g + 1) * TG],
                        asb[0:D, 0:TG], bc[0:D, 0:TG])

                    def pdrain(bi, tt, ec, ps):
                        # casts and store rings both alternate so the final
                        # eight stores drain two-wide (the single-ring issue
                        # serialization was the last ~2us of the teardown)
                        ysb = ysbp.tile([P, TG], BF16, name="ysb", tag="ysb")
                        if bi % 2:
                            nc.scalar.activation(
                                ysb, ps, mybir.ActivationFunctionType.Copy)
                            dq = nc.scalar
                        else:
                            nc.vector.tensor_copy(ysb, ps)
                            dq = nc.sync
                        dq.dma_start(
                            out=yp[tt * P:(tt + 1) * P,
                                   ec * TG:(ec + 1) * TG], in_=ysb)

                    for bi, (tt, ec, ps) in enumerate(tp):
                        nc.tensor.matmul(
                            ps, lhsT=ot[1][:, tt * P:(tt + 1) * P],
                            rhs=wp[1][:, ec * TG:(ec + 1) * TG],
                            start=False, stop=True, skip_group_check=True)
                        pdrain(bi, tt, ec, ps)
                    # last 2 units had no free PSUM bank for an early open
                    # (bcps holds 2 banks); run them as plain pairs
                    for bi, (tt, ec) in enumerate(units[6:]):
                        ps = mmp.tile([P, TG], F32, name="mm", tag="mm")
                        for i2 in range(2):
                            nc.tensor.matmul(
                                ps, lhsT=ot[i2][:, tt * P:(tt + 1) * P],
                                rhs=wp[i2][:, ec * TG:(ec + 1) * TG],
                                start=(i2 == 0), stop=(i2 == 1))
                        pdrain(bi, tt, ec, ps)
                    return None

                nc.gpsimd.dma_start(out=rcd_raw[slot], in_=asb[D:D + 1, :])
                rt = rtp.tile([P, 8], BF16, name="rt", tag="rt")
                nc.gpsimd.dma_start(out=rt, in_=bass.AP(
                    tensor=rcd_raw.tensor, offset=rcd_raw[slot].offset,
                    ap=[[8, P], [1, 8]]))

                # Epilogue part B (deferred into the next block so the
                # bounce round-trips never stall this DVE/sync stream).
                def part_b():
                    rw = rtp.tile([P, 8], BF16, name="rw", tag="rw")
                    with nc.allow_low_precision(
                            reason="bf16 softmax denominators (~0.4% rel)"):
                        nc.vector.reciprocal(rw, rt)
                    nc.gpsimd.dma_start(out=bass.AP(
                        tensor=rcd_rcp.tensor, offset=rcd_rcp[slot].offset,
                        ap=[[8, P], [1, 8]]), in_=rw)
                    bc = bcp.tile([P, 2 * TG], BF16, name="bc", tag="bc")
                    nc.gpsimd.dma_start(out=bc[0:D, :], in_=bass.AP(
                        tensor=rcd_rcp.tensor, offset=rcd_rcp[slot].offset,
                        ap=[[0, D], [1, 2 * TG]]))
                    normalize(bc)

                return part_b

            # ---- emission schedule ----
            # Pair 0's first chains + V(0..3) run before its g=0 block; the
            # rest of QKV, pair-1 chains and the output projections are fed
            # through the filler queue into the attention blocks' spare PE
            # slots (attention is exp-paced on ScalarE).
            fq = FillerQueue()
            fq.add(qk_chain_units(0, 0))
            fq.flush()
            for j in range(NT if general_mask else 4):
                fq.add(v_chain_units(j))
            pend = None
            cur_late = None
            for g in range(NG):
                if g < NG - 1:
                    fq.add(qk_chain_units(0, g + 1))
                    nxt_late = None
                    if not general_mask:
                        nxt_late = FillerQueue()
                        for j in range(4 * (g + 1), 4 * (g + 2)):
                            nxt_late.add(v_chain_units(j))
                else:
                    fq.add(qk_chain_units(1, 0))
                    nxt_late = None
                pend = attn_block(0, g, fq, pending=pend, defer_av=(g == 0),
                                  late_fq=cur_late)
                fq.flush()
                cur_late = nxt_late
            for g in range(NG):
                if g < NG - 1:
                    fq.add_front(qk_chain_units(1, g + 1))
                pend = attn_block(1, g, fq, pending=pend,
                                  fast_recip=(g == NG - 1))
                fq.flush()
                if g < NG - 1:
                    for tt in range(4 * g, 4 * g + 4):
                        for ec in range(2):
                            fq.add(proj_units(tt, ec))
            if pend is not None:
                pend()
            fq.flush()

    nc.compile()
    return nc


@lru_cache(maxsize=4)
def _program(apply_kbias: bool, general_mask: bool) -> bass.Bass:
    return build_program(apply_kbias, general_mask)


def _host_prep(inputs):
    x = np.asarray(inputs["x"], np.float32)
    Wq = np.asarray(inputs["Wq"], np.float32)
    bq = np.asarray(inputs["bq"], np.float32)
    Wk = np.asarray(inputs["Wk"], np.float32)
    bk = np.asarray(inputs["bk"], np.float32)
    Wv = np.asarray(inputs["Wv"], np.float32)
    bv = np.asarray(inputs["bv"], np.float32)
    Wp = np.asarray(inputs["Wp"], np.float32)
    attn_mask = np.asarray(inputs["attn_mask"])
    valid = np.asarray(inputs["valid_input_mask"])

    tril = np.tril(np.ones((T, T), attn_mask.dtype))
    causal = all(np.array_equal(attn_mask[b], tril) for b in range(B))
    kbias_all = (valid.astype(np.float32) - 1.0) * 1e6  # [B, T]
    apply_kbias = bool((valid == 0).any())

    band = np.where(np.arange(P)[:, None] <= np.arange(P)[None, :],
                    np.float32(0.0), np.float32(NEG))

    # device-ready layouts (pre-gathered so every DMA line is >=1 KB
    # contiguous):
    #   xg[tg*P+p, c*TG+u]   = x[b][tg*TG+u, c*P+p]
    #   wqg[iw*P+p, c*P+m]   = Wq[sl][iw*P+m, c*P+p]   (same for wk)
    #   wvg[p, c*DD+q]       = Wv[sl][q, c*P+p]
    def xg_prep(xb):
        return np.ascontiguousarray(
            xb.reshape(NG, TG, CCH, P).transpose(0, 3, 2, 1)
            .reshape(NG * P, CCH * TG)).astype(ml_dtypes.bfloat16)

    def wqkg_prep(Wsl):
        return np.ascontiguousarray(
            Wsl.reshape(2, P, CCH, P).transpose(0, 3, 2, 1)
            .reshape(2 * P, CCH * P)).astype(ml_dtypes.bfloat16)

    def wvg_prep(Wsl):
        return np.ascontiguousarray(
            Wsl.reshape(DD, CCH, P).transpose(2, 1, 0)
            .reshape(P, CCH * DD)).astype(ml_dtypes.bfloat16)

    in_maps = []
    for core in range(NCORES):
        b, hg = divmod(core, 4)
        sl = slice(hg * DD, (hg + 1) * DD)
        m = {
            "xg": xg_prep(x[b]),
            "wqg": wqkg_prep(Wq[sl, :]),
            "wkg": wqkg_prep(Wk[sl, :]),
            "wvg": wvg_prep(Wv[sl, :]),
            "wpT": np.ascontiguousarray(Wp[:, sl].T).astype(ml_dtypes.bfloat16),
            "bqk": np.ascontiguousarray(
                np.stack([bq[sl][:P], bq[sl][P:], bk[sl][:P], bk[sl][P:]], 1)),
            "bv_sb": np.ascontiguousarray(np.tile(bv[sl], (P, 1))),
            "ident": np.eye(D, dtype=ml_dtypes.bfloat16),
        }
        if apply_kbias:
            m["kbias"] = np.ascontiguousarray(kbias_all[b].reshape(NT, P).T)
        if not causal:
            m["maskT"] = np.ascontiguousarray(
                (attn_mask[b].T.astype(np.float32) - 1.0) * (-NEG))
        else:
            m["band"] = band
        in_maps.append(m)
    return in_maps, apply_kbias, causal


def _run(inputs, trace=False, trace_cores=None):
    global LAST_RESULTS
    in_maps, apply_kbias, causal = _host_prep(inputs)
    nc = _program(apply_kbias, not causal)
    res = bass_utils.run_bass_kernel_spmd(
        nc, in_maps, core_ids=list(range(NCORES)), trace=trace,
        trace_cores=trace_cores)
    LAST_RESULTS = res

    bp = np.asarray(inputs["bp"], np.float32)
    y = np.zeros((B, T, C), np.float32)
    for core in range(NCORES):
        y[core // 4] += np.asarray(res.results[core]["yp"], np.float32)
    y += bp[None, None, :]
    return y


def kernel(**inputs) -> np.ndarray:
    return _run(inputs)

